# revision 15
# baseline (speedup 1.0000x reference)
"""Causal self-attention (QK-RMSNorm + RoPE) Trainium2 kernel.

Sharding (Megatron-style, per the TP-over-heads hint):
  8 cores = 2 (batch) x 4 (head groups of 4 heads).
  Each core computes qkv/attention for its 4 heads on its batch and a partial
  projection output; the host sums the 4 partials per batch (the "all-reduce")
  and transposes (the device emits the output feature-major).

Per-core pipeline (all matmuls bf16 with fp32 PSUM accumulation):
  phase 0: cast x f32->bf16 (SWDGE cast DMA), bounce through DRAM in T/4
           quarters, DMA-transpose each quarter to x^T tiles
  phase 1: qkv = W_shard @ x^T (token-major PSUM) in pass order v, q, k;
           for q/k: fused RMSNorm (fp32 stats off PSUM) + RoPE (norm weights
           pre-folded into the rope tables), PE-transpose to [d, t] layout
  phase 2: per (q-block j, head): scores^T = k^T.T @ q^T, exp on ACT with
           tiles paired into 2-bank PSUM (no max subtraction needed:
           |scores| <= sqrt(hd)), causal mask by tile skipping + 4 diagonal
           masks, y^T = v.T @ p^T, denominator via DVE accumulate + fp32
           ones-matmul + DRAM-bounce broadcast; after each j-block's 4 heads,
           the projection for that block runs (overlaps next j's attention)
"""

import math
from contextlib import ExitStack

import numpy as np
import ml_dtypes

import concourse.bass as bass
import concourse.mybir as mybir
import concourse.tile as tile
from concourse import bacc

F32 = mybir.dt.float32
BF16 = mybir.dt.bfloat16
AF = mybir.ActivationFunctionType

# Problem constants (hardcoded; kernel.py must be self-contained)
B, T, C, H, HD = 2, 2048, 2048, 16, 128
N_CORES = 8
DP = 2                 # data-parallel ways (batch)
TPW = N_CORES // DP    # tensor-parallel ways (head groups)
HG = H // TPW          # heads per core
EPS = 1e-6


def build_nc(T_=T, C_=C, HG_=HG, hd=HD, TQ=512, TSPLIT=4):
    NT = T_ // 128       # token tiles
    NCt = C_ // 128      # contraction tiles for qkv
    NJ = T_ // TQ        # query-block tiles
    NO = C_ // 128       # output feature tiles
    R = TQ // 128        # diagonal mask patterns per query block
    F1 = HG_ * hd        # width of one of q/k/v chunks on this core
    HB = hd // 2
    NTS = NT // TSPLIT   # token tiles per staging quarter
    TL = T_ // TSPLIT    # tokens per staging quarter
    sm_scale = 1.0 / math.sqrt(hd)

    nc = bacc.Bacc(None, target_bir_lowering=False)
    x = nc.dram_tensor("x", [T_, C_], F32, kind="ExternalInput")
    wqkvT = nc.dram_tensor("wqkvT", [C_, 3 * F1], BF16, kind="ExternalInput")
    wprojT = nc.dram_tensor("wprojT", [F1, C_], BF16, kind="ExternalInput")
    rope_q = nc.dram_tensor("rope_q", [T_, 4 * HB], BF16, kind="ExternalInput")
    rope_k = nc.dram_tensor("rope_k", [T_, 4 * HB], BF16, kind="ExternalInput")
    masks_d = nc.dram_tensor("masks", [R * 128, TQ], BF16, kind="ExternalInput")
    ident_d = nc.dram_tensor("ident", [128, 128], BF16, kind="ExternalInput")
    outT = nc.dram_tensor("outT", [C_, T_], F32, kind="ExternalOutput")

    with tile.TileContext(nc) as tc, ExitStack() as big:
        persist = big.enter_context(tc.tile_pool(name="persist", bufs=1))
        dram = big.enter_context(tc.tile_pool(name="dram", bufs=1, space="DRAM"))

        v_all = persist.tile([128, NT, F1], BF16, tag="v_all")
        qkT = persist.tile([128, 2, HG_, T_], BF16, tag="qkT")
        ident = persist.tile([128, 128], BF16, tag="ident")
        nc.sync.dma_start(ident, ident_d[:])
        ones_f = persist.tile([128, 1], F32, tag="ones")
        nc.vector.memset(ones_f, 1.0)
        eps_t = persist.tile([128, 1], F32, tag="eps")
        nc.vector.memset(eps_t, EPS)

        # ---------------- phase 0 + 1: x^T staging and QKV ----------------
        with ExitStack() as ph1:
            xh_pool = ph1.enter_context(tc.tile_pool(name="xT", bufs=1))
            stage_pool = ph1.enter_context(tc.tile_pool(name="stage", bufs=2))
            wt_pool = ph1.enter_context(tc.tile_pool(name="wt", bufs=2))
            rope_pool = ph1.enter_context(tc.tile_pool(name="rope", bufs=1))
            scr = ph1.enter_context(tc.tile_pool(name="qkscr", bufs=2))
            ps_qkv = ph1.enter_context(
                tc.tile_pool(name="ps_qkv", bufs=4, space="PSUM"))
            ps_tr = ph1.enter_context(
                tc.tile_pool(name="ps_tr", bufs=3, space="PSUM"))

            rope_sb = {}
            for nm, dr in (("q", rope_q), ("k", rope_k)):
                t_ = rope_pool.tile([128, NT, 4 * HB], BF16, tag=f"rope{nm}")
                nc.gpsimd.dma_start(t_, dr[:].rearrange("(n p) f -> p n f", p=128))
                rope_sb[nm] = t_

            # W chunks load on the sync HWDGE ring (bf16, no cast needed)
            def load_wt(fc):
                wt = wt_pool.tile([128, NCt, F1], BF16, tag="wt")
                for ci in range(NCt):
                    nc.sync.dma_start(
                        wt[:, ci, :],
                        wqkvT[ci * 128:(ci + 1) * 128, fc * F1:(fc + 1) * F1],
                    )
                return wt

            wts = {2: load_wt(2)}
            xhs = [None] * NT

            for fc in (2, 0, 1):  # pass order: v, q, k
                wt = wts.get(fc)
                if wt is None:
                    wt = load_wt(fc)
                for i in range(NT):
                    if fc == 2:
                        # x^T production fused into the v pass: SWDGE
                        # cast-load the x tile, PE-transpose each 128x128
                        # block, then run the v accumulation group
                        xb = stage_pool.tile([128, C_], BF16, tag="stage")
                        nc.gpsimd.dma_start(xb, x[i * 128:(i + 1) * 128, :])
                        xhi = xh_pool.tile([128, NCt, 128], BF16, tag=f"xh{i}")
                        for cq in range(NCt // 4):
                            pt = ps_tr.tile([128, 512], BF16, tag="pstr")
                            for c4 in range(4):
                                ci = 4 * cq + c4
                                nc.tensor.transpose(
                                    pt[:, c4 * 128:(c4 + 1) * 128],
                                    xb[:, ci * 128:(ci + 1) * 128], ident)
                            dst = xhi[:, 4 * cq:4 * cq + 4, :]
                            if cq % 2 == 0:
                                nc.vector.tensor_copy(dst, pt)
                            else:
                                nc.scalar.copy(dst, pt)
                        xhs[i] = xhi
                    ps = ps_qkv.tile([128, F1], F32, tag="psqkv")
                    for ci in range(NCt):
                        nc.tensor.matmul(
                            ps,
                            xhs[i][:, ci, :],
                            wt[:, ci, :],
                            start=(ci == 0),
                            stop=(ci == NCt - 1),
                        )
                    if fc == 2:
                        nc.vector.tensor_copy(v_all[:, i, :], ps)
                        continue
                    rp = rope_sb["q" if fc == 0 else "k"]
                    # RMS norm stats in fp32 off PSUM (ACT: square+accum, sqrt)
                    sq = scr.tile([128, F1], BF16, tag="sq")
                    ssq = scr.tile([128, HG_], F32, tag="ssq")
                    for h in range(HG_):
                        nc.scalar.activation(
                            sq[:, h * hd:(h + 1) * hd],
                            ps[:, h * hd:(h + 1) * hd],
                            AF.Square,
                            accum_out=ssq[:, h:h + 1],
                        )
                    sstd = scr.tile([128, HG_], F32, tag="sstd")
                    nc.scalar.activation(
                        sstd, ssq, AF.Sqrt, bias=eps_t[:, 0:1], scale=1.0 / hd
                    )
                    rinv = scr.tile([128, HG_], F32, tag="rinv")
                    nc.vector.reciprocal(rinv, sstd)
                    qn = scr.tile([128, F1], BF16, tag="qn")
                    for h in range(HG_):
                        nc.vector.tensor_scalar_mul(
                            out=qn[:, h * hd:(h + 1) * hd],
                            in0=ps[:, h * hd:(h + 1) * hd],
                            scalar1=rinv[:, h:h + 1],
                        )
                    # RoPE (norm weights pre-folded into the rope tables)
                    qn3 = qn.rearrange("p (h two d) -> p h two d", h=HG_, two=2)
                    rq = scr.tile([128, F1], BF16, tag="rq")
                    rq3 = rq.rearrange("p (h two d) -> p h two d", h=HG_, two=2)
                    tmp = scr.tile([128, HG_ * HB], BF16, tag="tmp")
                    tm3 = tmp.rearrange("p (h d) -> p h d", h=HG_)
                    tmp2 = scr.tile([128, HG_ * HB], BF16, tag="tmp2")
                    tm23 = tmp2.rearrange("p (h d) -> p h d", h=HG_)

                    def rope_c(c_idx):
                        bse = rp[:, i, c_idx * HB:(c_idx + 1) * HB]
                        return bass.AP(
                            tensor=bse.tensor,
                            offset=bse.offset,
                            ap=[list(bse.ap[0]), [0, HG_], list(bse.ap[-1])],
                        )

                    nc.vector.tensor_mul(tm3, qn3[:, :, 0, :], rope_c(0))
                    nc.vector.tensor_mul(tm23, qn3[:, :, 1, :], rope_c(1))
                    nc.vector.tensor_sub(rq3[:, :, 0, :], tm3, tm23)
                    nc.vector.tensor_mul(tm3, qn3[:, :, 1, :], rope_c(2))
                    nc.vector.tensor_mul(tm23, qn3[:, :, 0, :], rope_c(3))
                    nc.vector.tensor_add(rq3[:, :, 1, :], tm3, tm23)
                    # head-transpose q/k (4 heads into one PSUM bank, 1 copy)
                    pt = ps_tr.tile([128, HG_ * hd], BF16, tag="pstr")
                    for h in range(HG_):
                        nc.tensor.transpose(
                            pt[:, h * hd:(h + 1) * hd],
                            rq[:, h * hd:(h + 1) * hd], ident)
                    dst = qkT[:, fc, :, i * 128:(i + 1) * 128]
                    ptv = pt.rearrange("p (h t) -> p h t", h=HG_)
                    if i % 2 == 0:
                        nc.vector.tensor_copy(dst, ptv)
                    else:
                        nc.scalar.copy(dst, ptv)

        # -------- phase 2 + 3: attention with interleaved projection --------
        with ExitStack() as ph2:
            pP = ph2.enter_context(tc.tile_pool(name="pP", bufs=10))
            dP = ph2.enter_context(tc.tile_pool(name="dP", bufs=2))
            rbP = ph2.enter_context(tc.tile_pool(name="rbP", bufs=2))
            yP = ph2.enter_context(tc.tile_pool(name="yP", bufs=1))
            wpP = ph2.enter_context(tc.tile_pool(name="wpP", bufs=1))
            oP = ph2.enter_context(tc.tile_pool(name="oP", bufs=3))
            ps_s = ph2.enter_context(tc.tile_pool(name="ps_s", bufs=2, space="PSUM"))
            ps_yp = ph2.enter_context(tc.tile_pool(name="ps_y", bufs=2, space="PSUM"))
            ps_m = ph2.enter_context(tc.tile_pool(name="ps_m", bufs=1, space="PSUM"))

            masks_t = wpP.tile([128, R * TQ], BF16, tag="masks")
            for r in range(R):
                nc.sync.dma_start(
                    masks_t[:, r * TQ:(r + 1) * TQ],
                    masks_d[r * 128:(r + 1) * 128, :]
                )
            wp = wpP.tile([128, HG_, C_], BF16, tag="wp")
            for ci in range(HG_):
                nc.gpsimd.dma_start(wp[:, ci, :], wprojT[ci * 128:(ci + 1) * 128, :])

            for j in range(NJ):
                yTj = yP.tile([128, HG_, TQ], BF16, tag=f"yT{j}", bufs=1)
                nk = R * j + R          # valid 128-wide k tiles (causal)
                npairs = nk // 2
                for h in range(HG_):
                    dacc2 = dP.tile([128, 2 * TQ], F32, tag="dacc2")
                    plist = []
                    for kp in range(npairs):
                        s2 = ps_s.tile([128, 2 * TQ], F32, tag="s2")
                        for half in range(2):
                            k = 2 * kp + half
                            nc.tensor.matmul(
                                s2[:, half * TQ:(half + 1) * TQ],
                                qkT[:, 1, h, k * 128:(k + 1) * 128],
                                qkT[:, 0, h, j * TQ:(j + 1) * TQ],
                                start=True,
                                stop=True,
                            )
                        p2 = pP.tile([128, 2 * TQ], BF16, tag="p2")
                        nc.scalar.activation(p2, s2, AF.Exp, scale=sm_scale)
                        if kp >= npairs - 2:  # the two diagonal-block pairs
                            r0 = 2 * kp - R * j
                            nc.vector.tensor_mul(
                                p2, p2, masks_t[:, r0 * TQ:(r0 + 2) * TQ]
                            )
                        if kp == 0:
                            nc.vector.tensor_copy(dacc2, p2)
                        else:
                            nc.vector.tensor_add(dacc2, dacc2, p2)
                        plist.append(p2)
                    yps = ps_yp.tile([128, TQ], F32, tag="yps")
                    for k in range(nk):
                        nc.tensor.matmul(
                            yps,
                            v_all[:, k, h * hd:(h + 1) * hd],
                            plist[k // 2][:, (k % 2) * TQ:(k % 2 + 1) * TQ],
                            start=(k == 0),
                            stop=(k == nk - 1),
                        )
                    dsum = dP.tile([128, TQ], F32, tag="dsum")
                    nc.vector.tensor_add(dsum, dacc2[:, :TQ], dacc2[:, TQ:])
                    # partition-dim reduction via fp32 ones-matmul
                    psr = ps_m.tile([1, TQ], F32, tag="psr")
                    nc.tensor.matmul(psr, ones_f[:, 0:1], dsum, start=True, stop=True)
                    row = dP.tile([1, TQ], F32, tag="row")
                    nc.vector.reciprocal_approx_fast(row, psr)
                    # broadcast the reciprocal row across partitions via DRAM
                    rowd = dram.tile([TQ], F32, tag="rowd", bufs=2)
                    nc.sync.dma_start(rowd, row)
                    rb = rbP.tile([128, TQ], F32, tag="rb")
                    nc.gpsimd.dma_start(
                        rb,
                        bass.AP(tensor=rowd.tensor, offset=rowd.offset,
                                ap=[[0, 128], list(rowd.ap[-1])]),
                    )
                    nc.vector.tensor_mul(yTj[:, h, :], yps, rb)
                # projection for this q block (overlaps next block's attention)
                for o in range(NO):
                    pp = ps_m.tile([128, TQ], F32, tag="pp")
                    for ci in range(HG_):
                        nc.tensor.matmul(
                            pp,
                            wp[:, ci, o * 128:(o + 1) * 128],
                            yTj[:, ci, :],
                            start=(ci == 0),
                            stop=(ci == HG_ - 1),
                        )
                    ost = oP.tile([128, TQ], F32, tag="ost")
                    nc.scalar.copy(ost, pp)
                    nc.sync.dma_start(
                        outT[o * 128:(o + 1) * 128, j * TQ:(j + 1) * TQ], ost
                    )

    nc.compile()
    return nc


def make_host_inputs(x, Wqkv, Wproj, q_norm_w, k_norm_w, rope_cos, rope_sin,
                     T_=T, C_=C, HG_=HG, hd=HD, TQ=512):
    """Build the 8 per-core input maps (sharding done on host)."""
    H_ = Wqkv.shape[0] // (3 * hd)
    tpw = H_ // HG_
    R = TQ // 128
    HB = hd // 2

    def rope_tables(w):
        # out1 = qn1*(cos*w1) - qn2*(sin*w2); out2 = qn2*(cos*w2) + qn1*(sin*w1)
        w1, w2 = w[:HB], w[HB:]
        A = rope_cos * w1[None, :]
        Bt = rope_sin * w2[None, :]
        Ct = rope_cos * w2[None, :]
        D = rope_sin * w1[None, :]
        return np.ascontiguousarray(
            np.concatenate([A, Bt, Ct, D], axis=1).astype(ml_dtypes.bfloat16)
        )

    rope_q_h = rope_tables(np.asarray(q_norm_w, dtype=np.float32))
    rope_k_h = rope_tables(np.asarray(k_norm_w, dtype=np.float32))

    # diagonal causal masks: pattern r: valid when tk + 128*r <= tq
    tk = np.arange(128)[:, None]
    tq = np.arange(TQ)[None, :]
    masks = np.concatenate(
        [(tk + 128 * r <= tq) for r in range(R)], axis=0
    ).astype(ml_dtypes.bfloat16)

    Wqkv = np.asarray(Wqkv, dtype=np.float32)
    Wproj = np.asarray(Wproj, dtype=np.float32)
    x = np.asarray(x, dtype=np.float32)

    in_maps = []
    for core in range(N_CORES):
        b = core // tpw
        g = core % tpw
        rs = slice(g * HG_ * hd, (g + 1) * HG_ * hd)
        W_shard = np.concatenate(
            [Wqkv[0 * H_ * hd:][rs.start:rs.stop],
             Wqkv[1 * H_ * hd:][rs.start:rs.stop],
             Wqkv[2 * H_ * hd:][rs.start:rs.stop]], axis=0
        )  # [3*F1, C]
        in_maps.append({
            "ident": np.eye(128, dtype=ml_dtypes.bfloat16),
            "x": np.ascontiguousarray(x[b]),
            "wqkvT": np.ascontiguousarray(W_shard.T).astype(ml_dtypes.bfloat16),
            "wprojT": np.ascontiguousarray(Wproj[:, rs].T).astype(ml_dtypes.bfloat16),
            "rope_q": rope_q_h,
            "rope_k": rope_k_h,
            "masks": masks,
        })
    return in_maps


_NC_CACHE = {}


def run_spmd(inputs, **run_kwargs):
    from concourse.bass_utils import run_bass_kernel_spmd

    x = np.asarray(inputs["x"])
    in_maps = make_host_inputs(
        x, inputs["Wqkv"], inputs["Wproj"], inputs["q_norm_w"],
        inputs["k_norm_w"], inputs["rope_cos"], inputs["rope_sin"],
    )
    if "nc" not in _NC_CACHE:
        _NC_CACHE["nc"] = build_nc()
    nc = _NC_CACHE["nc"]
    res = run_bass_kernel_spmd(nc, in_maps, core_ids=list(range(N_CORES)),
                               **run_kwargs)
    tpw = N_CORES // B
    out = np.zeros((B, T, C), dtype=np.float32)
    for core in range(N_CORES):
        b = core // tpw
        out[b] += res.results[core]["outT"].T
    return out, res


def kernel(**inputs):
    out, _ = run_spmd(inputs)
    return out
